# revision 18
# baseline (speedup 1.0000x reference)
"""Trainium2 Bass kernel for AdjStackAttentionWeights (masked BN + MLP over edge stacks).

Reference: stacks [64,256,256,16] f32, mask [64,256] bool (True = padded row).
BN(train) with masked batch stats over S=16 channels, then Linear(16->32) +
ReLU + Linear(32->8), masked rows zeroed. Output [64,256,256,8] f32.

Sharding: data-parallel over b across 8 NeuronCores (8 graphs/core,
R = 524288 rows x 16 chans per core).

Per core (single data read):
  Pass 1: SWDGE cast-DMA streams the whole shard into SBUF as bf16
    (16.8 MB, retained), computes masked sum / sum-of-squares via TensorE
    (lhsT = per-partition mask weight column), AllReduce of 33 floats.
  BN fold: W1' = diag(scale) @ W1 (block-diag, bf16), b1' = b1 + shift@W1.
  Pass 2: reads the retained bf16 tiles; PE-transpose pairs of sub-tiles to
    channel-major, block-diag matmuls for both MLP layers (layer 1 split
    into lo/hi partition-halves), PE-transpose back, +b2, mask, store.
"""

import sys
import numpy as np

_REPO = "/opt/trn_rl_repo"
if _REPO not in sys.path:
    sys.path.insert(0, _REPO)

import ml_dtypes  # noqa: E402
from concourse import bass, bacc, tile  # noqa: E402
from concourse.bass_utils import run_bass_kernel_spmd  # noqa: E402

mybir = bass.mybir
F32 = mybir.dt.float32
BF16 = mybir.dt.bfloat16
BF_NP = ml_dtypes.bfloat16

B, NN, S, HID, HEADS = 64, 256, 16, 32, 8
NCORES = 8
BL = B // NCORES
R = BL * NN * NN                 # 524288 rows per core
NT2 = 256                        # logical tiles [128, 256] (2048 rows)
NL2 = NT2 // 2                   # paired loads [128, 512]
EPS = 1e-5


def build():
    nc = bacc.Bacc("TRN2", target_bir_lowering=False, debug=False,
                   num_devices=NCORES)

    x_t = nc.dram_tensor("stacks", [R, S], F32, kind="ExternalInput")
    wcol2_t = nc.dram_tensor("wcol2", [128, NT2], F32, kind="ExternalInput")
    wc2b_t = nc.dram_tensor("wc2b", [128, NT2], BF16, kind="ExternalInput")
    onesb_t = nc.dram_tensor("onesb", [128, 1], BF16, kind="ExternalInput")
    w1bd_t = nc.dram_tensor("w1bd", [64, 128], F32, kind="ExternalInput")
    w1lo_t = nc.dram_tensor("w1lo", [128, 128], F32, kind="ExternalInput")
    w1hi_t = nc.dram_tensor("w1hi", [128, 128], F32, kind="ExternalInput")
    w2bd_t = nc.dram_tensor("w2bd", [128, 32], BF16, kind="ExternalInput")
    identb_t = nc.dram_tensor("identb", [128, 128], BF16, kind="ExternalInput")
    pcon_t = nc.dram_tensor("pcon", [128, 4], F32, kind="ExternalInput")
    rcon_t = nc.dram_tensor("rcon", [1, 32], F32, kind="ExternalInput")
    out_t = nc.dram_tensor("out", [R, HEADS], F32, kind="ExternalOutput")

    # paired loads: row = 2048*(2L+tt) + 16*i + f ; free = (tt,f,s)
    xp2 = x_t.ap().rearrange("(T tt i f) s -> T i tt (f s)",
                             T=NL2, tt=2, i=128, f=16)
    outv = out_t.ap().rearrange("(T tt i f) k -> T i tt (f k)",
                                T=NL2, tt=2, i=128, f=16)

    AF = mybir.ActivationFunctionType
    AL = mybir.AluOpType

    with tile.TileContext(nc) as tc:
        with tc.tile_pool(name="consts", bufs=1) as cp, \
             tc.tile_pool(name="persist", bufs=1) as pp, \
             tc.tile_pool(name="keep", bufs=1) as kp, \
             tc.tile_pool(name="dram", bufs=1, space="DRAM") as dp:

            w1bd = cp.tile([64, 128], F32)
            w1lo = cp.tile([128, 128], F32)
            w1hi = cp.tile([128, 128], F32)
            w2bd = cp.tile([128, 32], BF16)
            identb = cp.tile([128, 128], BF16)
            wc2 = cp.tile([128, NT2], F32)
            wc2b = cp.tile([128, NT2], BF16)
            onesb = cp.tile([128, 1], BF16)
            pcon = cp.tile([128, 4], F32)
            rcon = cp.tile([1, 32], F32)
            nc.sync.dma_start(out=w1bd[:], in_=w1bd_t.ap())
            nc.sync.dma_start(out=w1lo[:], in_=w1lo_t.ap())
            nc.sync.dma_start(out=w1hi[:], in_=w1hi_t.ap())
            nc.sync.dma_start(out=w2bd[:], in_=w2bd_t.ap())
            nc.sync.dma_start(out=identb[:], in_=identb_t.ap())
            nc.sync.dma_start(out=wc2[:], in_=wcol2_t.ap())
            nc.sync.dma_start(out=wc2b[:], in_=wc2b_t.ap())
            nc.sync.dma_start(out=onesb[:], in_=onesb_t.ap())
            nc.sync.dma_start(out=pcon[:], in_=pcon_t.ap())
            nc.sync.dma_start(out=rcon[:], in_=rcon_t.ap())

            pack = pp.tile([1, 64], F32)
            stats = pp.tile([1, 64], F32)
            sca = pp.tile([1, 128], F32)    # tile(scale, 8)
            scb = pp.tile([1, 64], F32)     # tile(shift, 4)
            scaleb8 = pp.tile([128, 1], F32)
            shiftb = pp.tile([64, 1], F32)
            w1lop = pp.tile([128, 128], BF16)
            w1hip = pp.tile([128, 128], BF16)
            b1p = pp.tile([128, 1], F32)
            st = pp.tile([1, 16], F32)
            st2 = pp.tile([1, 16], F32)
            st3 = pp.tile([1, 16], F32)
            rcnt = pp.tile([1, 1], F32)

            bncA = dp.tile([1, 64], F32)
            bncB = dp.tile([1, 64], F32)

            # retained bf16 shard (16.8 MB), one tile per paired load
            xk = [kp.tile([128, 512], BF16, tag=f"xk{L}", name=f"xk{L}")
                  for L in range(NL2)]

            # ============ pass 1: cast-load + masked stats ============
            with tc.tile_pool(name="p1s", bufs=2) as p1s, \
                 tc.tile_pool(name="p1p", bufs=1, space="PSUM") as p1p:
                psx = p1p.tile([1, 256], F32)
                psx2 = p1p.tile([1, 256], F32)
                pcnt = p1p.tile([1, 256], F32)
                for L in range(NL2):
                    # cast f32 -> bf16 during the (SWDGE) DMA
                    nc.gpsimd.dma_start(
                        out=xk[L][:].rearrange("p (tt g) -> p tt g",
                                               tt=2, g=256),
                        in_=xp2[L])
                    sq = p1s.tile([128, 512], BF16)
                    nc.vector.tensor_tensor(sq[:], xk[L][:], xk[L][:],
                                            op=AL.mult)
                    for tt in range(2):
                        t = 2 * L + tt
                        wcol = wc2b[:, t:t + 1]
                        nc.tensor.matmul(
                            psx[:], wcol, xk[L][:, 256 * tt:256 * tt + 256],
                            start=(t == 0), stop=(t == NT2 - 1),
                            skip_group_check=True)
                        nc.tensor.matmul(
                            psx2[:], wcol, sq[:, 256 * tt:256 * tt + 256],
                            start=(t == 0), stop=(t == NT2 - 1),
                            skip_group_check=True)
                nc.tensor.matmul(pcnt[:], onesb[:], wc2b[:],
                                 start=True, stop=True, skip_group_check=True)
                nc.vector.memset(pack[:], 0.0)
                # psum free layout f = 16*q + s: reduce over q keeping s
                nc.vector.tensor_reduce(
                    pack[0:1, 0:16],
                    psx.rearrange("p (q s) -> p s q", q=16, s=16),
                    axis=mybir.AxisListType.X, op=AL.add)
                nc.vector.tensor_reduce(
                    pack[0:1, 16:32],
                    psx2.rearrange("p (q s) -> p s q", q=16, s=16),
                    axis=mybir.AxisListType.X, op=AL.add)
                nc.vector.tensor_reduce(
                    pack[0:1, 32:33], pcnt[:],
                    axis=mybir.AxisListType.X, op=AL.add)

            # ================= all-reduce + BN fold =================
            nc.gpsimd.dma_start(out=bncA[:], in_=pack[:])
            nc.gpsimd.collective_compute(
                "AllReduce", AL.add,
                replica_groups=[list(range(NCORES))],
                ins=[bncA.opt()], outs=[bncB.opt()])
            nc.gpsimd.dma_start(out=stats[:], in_=bncB[:])

            s_sx = stats[0:1, 0:16]
            s_sx2 = stats[0:1, 16:32]
            s_cnt = stats[0:1, 32:33]
            gam = rcon[0:1, 0:16]
            bet = rcon[0:1, 16:32]
            nc.vector.reciprocal(rcnt[:], s_cnt)
            nc.vector.tensor_scalar_mul(st[:], s_sx, rcnt[:])    # mean
            nc.vector.tensor_scalar_mul(st2[:], s_sx2, rcnt[:])  # E[x^2]
            nc.vector.tensor_tensor(st3[:], st[:], st[:], op=AL.mult)
            nc.vector.tensor_sub(st2[:], st2[:], st3[:])         # var
            nc.vector.tensor_scalar_add(st2[:], st2[:], float(EPS))
            nc.vector.reciprocal(st3[:], st2[:])
            nc.scalar.sqrt(st3[:], st3[:])                       # rstd
            nc.vector.tensor_tensor(st3[:], st3[:], gam, op=AL.mult)   # scale
            nc.vector.tensor_tensor(st2[:], st[:], st3[:], op=AL.mult)
            nc.vector.tensor_sub(st2[:], bet, st2[:])            # shift
            for g in range(8):
                nc.vector.tensor_copy(sca[0:1, 16 * g:16 * g + 16], st3[:])
            for g in range(4):
                nc.vector.tensor_copy(scb[0:1, 16 * g:16 * g + 16], st2[:])
            nc.sync.dma_start(out=scaleb8[:], in_=sca[:])
            nc.sync.dma_start(out=shiftb[:], in_=scb[:])
            nc.scalar.mul(w1lop[:], w1lo[:], mul=scaleb8[:])
            nc.scalar.mul(w1hip[:], w1hi[:], mul=scaleb8[:])
            with tc.tile_pool(name="stp", bufs=1, space="PSUM") as stp:
                pb1 = stp.tile([128, 1], F32)
                nc.tensor.matmul(pb1[:], w1bd[:], shiftb[:],
                                 start=True, stop=True, skip_group_check=True)
                nc.scalar.activation(b1p[:], pb1[:], AF.Identity,
                                     bias=pcon[:, 1:2], scale=1.0)

            # ================= pass 2: MLP =================
            # logical tile t = 2L+tt; sub-tile j = 2a+jj; rows
            # r = 2048t + 16i + 4j + g, chan s.
            with tc.tile_pool(name="p2st", bufs=3) as p2st, \
                 tc.tile_pool(name="p2sh", bufs=3) as p2sh, \
                 tc.tile_pool(name="p2sy", bufs=3) as p2sy, \
                 tc.tile_pool(name="p2so", bufs=3) as p2so, \
                 tc.tile_pool(name="p2pa", bufs=2, space="PSUM") as p2pa, \
                 tc.tile_pool(name="p2pb", bufs=2, space="PSUM") as p2pb, \
                 tc.tile_pool(name="p2pc", bufs=2, space="PSUM") as p2pc, \
                 tc.tile_pool(name="p2pd", bufs=2, space="PSUM") as p2pd:
                for L in range(NL2):
                    xab = xk[L]
                    # 4 transposes: quarter q=(tt,a) -> pxT[:,128q]
                    pxT = p2pa.tile([128, 512], BF16)
                    for q in range(4):
                        nc.tensor.transpose(pxT[:, 128 * q:128 * q + 128],
                                            xab[:, 128 * q:128 * q + 128],
                                            identb[:])
                    sxT = p2st.tile([128, 512], BF16)
                    nc.scalar.copy(sxT[:], pxT[:])
                    py = p2pc.tile([128, 256], F32)
                    for tt in range(2):
                        # layer 1: lo covers jj=0 (partitions 0:64), hi jj=1
                        ph = p2pb.tile([128, 512], F32, tag="ph")
                        rhs = sxT[:, 256 * tt:256 * tt + 256]
                        nc.tensor.matmul(ph[:, 0:256], w1lop[:], rhs,
                                         start=True, stop=True,
                                         skip_group_check=True)
                        nc.tensor.matmul(ph[:, 256:512], w1hip[:], rhs,
                                         start=True, stop=True,
                                         skip_group_check=True)
                        sh = p2sh.tile([128, 512], BF16, tag="sh")
                        if tt == 0:
                            nc.scalar.activation(sh[:], ph[:], AF.Relu,
                                                 bias=b1p[:], scale=1.0)
                        else:
                            # bias-add then relu on DVE
                            nc.vector.tensor_scalar(
                                sh[:], ph[:], b1p[:], 0.0,
                                op0=AL.add, op1=AL.max)
                        # layer 2: sub-tile j=2a+jj at sh[:, 256jj+128a]
                        for jj in range(2):
                            for a in range(2):
                                j = 2 * a + jj
                                nc.tensor.matmul(
                                    py[32 * j:32 * j + 32,
                                       128 * tt:128 * tt + 128],
                                    w2bd[:],
                                    sh[:, 256 * jj + 128 * a:
                                       256 * jj + 128 * a + 128],
                                    start=True, stop=True,
                                    skip_group_check=True,
                                    tile_position=(0, 32 * j))
                    sy = p2sy.tile([128, 256], BF16)
                    nc.vector.tensor_scalar_add(sy[:], py[:], pcon[:, 2:3])
                    pyr = p2pd.tile([128, 256], BF16)
                    for u in range(2):
                        nc.tensor.transpose(pyr[:, 128 * u:128 * u + 128],
                                            sy[:, 128 * u:128 * u + 128],
                                            identb[:])
                    so = p2so.tile([128, 256], F32)
                    nc.scalar.mul(so[:, 0:128], pyr[:, 0:128],
                                  mul=wc2[:, 2 * L:2 * L + 1])
                    nc.vector.tensor_scalar_mul(
                        so[:, 128:256], pyr[:, 128:256],
                        wc2[:, 2 * L + 1:2 * L + 2])
                    nc.sync.dma_start(
                        out=outv[L],
                        in_=so[:].rearrange("p (tt g) -> p tt g",
                                            tt=2, g=128))

    nc.compile()
    return nc


_NC = None


def _get_nc():
    global _NC
    if _NC is None:
        _NC = build()
    return _NC


def _prep_inputs(stacks, mask, gamma, beta, W1, b1, W2, b2):
    stacks = np.ascontiguousarray(np.asarray(stacks, dtype=np.float32))
    mask = np.asarray(mask)
    w_full = (~mask.astype(bool)).astype(np.float32)
    gamma = np.asarray(gamma, np.float32)
    beta = np.asarray(beta, np.float32)
    W1 = np.asarray(W1, np.float32)
    b1 = np.asarray(b1, np.float32)
    W2 = np.asarray(W2, np.float32)
    b2 = np.asarray(b2, np.float32)

    w1bd = np.zeros((64, 128), np.float32)
    w1lo = np.zeros((128, 128), np.float32)
    w1hi = np.zeros((128, 128), np.float32)
    w2bd = np.zeros((128, 32), np.float32)
    for g in range(4):
        w1bd[16 * g:16 * g + 16, 32 * g:32 * g + 32] = W1
        w1lo[16 * g:16 * g + 16, 32 * g:32 * g + 32] = W1
        w1hi[64 + 16 * g:64 + 16 * g + 16, 32 * g:32 * g + 32] = W1
        w2bd[32 * g:32 * g + 32, 8 * g:8 * g + 8] = W2
    identb = np.eye(128).astype(BF_NP)
    onesb = np.full((128, 1), 16.0, BF_NP)   # 16 rows per wcol2 entry
    pcon = np.zeros((128, 4), np.float32)
    pcon[:, 1] = np.tile(b1, 4)
    pcon[:, 2] = np.tile(b2, 16)
    rcon = np.zeros((1, 32), np.float32)
    rcon[0, 0:16] = gamma
    rcon[0, 16:32] = beta

    I2 = np.arange(128) // 16
    T2 = 8 * np.arange(NT2)

    in_maps = []
    for c in range(NCORES):
        shard = stacks[c * BL:(c + 1) * BL].reshape(R, S)
        w = w_full[c * BL:(c + 1) * BL].reshape(BL * NN)
        wcol2 = np.ascontiguousarray(
            w[I2[:, None] + T2[None, :]].astype(np.float32))
        in_maps.append({
            "stacks": shard,
            "wcol2": wcol2, "wc2b": wcol2.astype(BF_NP), "onesb": onesb,
            "w1bd": w1bd, "w1lo": w1lo, "w1hi": w1hi,
            "w2bd": w2bd.astype(BF_NP), "identb": identb,
            "pcon": pcon, "rcon": rcon,
        })
    return in_maps


def _run(inputs, trace=False, tmpdir=None):
    nc = _get_nc()
    in_maps = _prep_inputs(**inputs)
    res = run_bass_kernel_spmd(nc, in_maps, core_ids=list(range(NCORES)),
                               trace=trace, tmpdir=tmpdir)
    outs = [res.results[c]["out"].reshape(BL, NN, NN, HEADS)
            for c in range(NCORES)]
    full = np.concatenate(outs, axis=0).astype(np.float32)
    return full, res


def kernel(**inputs):
    out, _ = _run(inputs, trace=False)
    return out


# revision 23
# speedup vs baseline: 1.2144x; 1.2144x over previous
"""Trainium2 Bass kernel for AdjStackAttentionWeights (masked BN + MLP over edge stacks).

Reference: stacks [64,256,256,16] f32, mask [64,256] bool (True = padded row).
BN(train) with masked batch stats over S=16 channels, then Linear(16->32) +
ReLU + Linear(32->8), masked rows zeroed. Output [64,256,256,8] f32.

Sharding: data-parallel over b across 8 NeuronCores (8 graphs/core,
R = 524288 rows x 16 chans per core).

Per core (single data read):
  Pass 1: SWDGE cast-DMA streams the whole shard into SBUF as bf16
    (16.8 MB, retained), computes masked sum / sum-of-squares via TensorE
    (lhsT = per-partition mask weight column), AllReduce of 33 floats.
  BN fold: W1' = diag(scale) @ W1 (block-diag, bf16), b1' = b1 + shift@W1.
  Pass 2: reads the retained bf16 tiles; PE-transpose pairs of sub-tiles to
    channel-major, block-diag matmuls for both MLP layers (layer 1 split
    into lo/hi partition-halves), PE-transpose back, +b2, mask, store.
"""

import sys
import numpy as np

_REPO = "/opt/trn_rl_repo"
if _REPO not in sys.path:
    sys.path.insert(0, _REPO)

import ml_dtypes  # noqa: E402
from concourse import bass, bacc, tile  # noqa: E402
from concourse.bass_utils import run_bass_kernel_spmd  # noqa: E402

mybir = bass.mybir
F32 = mybir.dt.float32
BF16 = mybir.dt.bfloat16
BF_NP = ml_dtypes.bfloat16

B, NN, S, HID, HEADS = 64, 256, 16, 32, 8
NCORES = 8
BL = B // NCORES
R = BL * NN * NN                 # 524288 rows per core
NT2 = 256                        # logical tiles [128, 256] (2048 rows)
NL2 = NT2 // 2                   # paired loads [128, 512]
EPS = 1e-5


def build():
    nc = bacc.Bacc("TRN2", target_bir_lowering=False, debug=False,
                   num_devices=NCORES)

    x_t = nc.dram_tensor("stacks", [R, S], F32, kind="ExternalInput")
    wcol2_t = nc.dram_tensor("wcol2", [128, NT2], F32, kind="ExternalInput")
    wc2b_t = nc.dram_tensor("wc2b", [128, NT2], BF16, kind="ExternalInput")
    onesb_t = nc.dram_tensor("onesb", [128, 1], BF16, kind="ExternalInput")
    w1bd_t = nc.dram_tensor("w1bd", [64, 128], F32, kind="ExternalInput")
    w1lo_t = nc.dram_tensor("w1lo", [128, 128], F32, kind="ExternalInput")
    w1hi_t = nc.dram_tensor("w1hi", [128, 128], F32, kind="ExternalInput")
    w2bd_t = nc.dram_tensor("w2bd", [128, 32], BF16, kind="ExternalInput")
    identb_t = nc.dram_tensor("identb", [128, 128], BF16, kind="ExternalInput")
    pcon_t = nc.dram_tensor("pcon", [128, 4], F32, kind="ExternalInput")
    rcon_t = nc.dram_tensor("rcon", [1, 32], F32, kind="ExternalInput")
    out_t = nc.dram_tensor("out", [R, HEADS], F32, kind="ExternalOutput")

    # paired loads: row = 2048*(2L+tt) + 16*i + f ; free = (tt,f,s)
    xp2 = x_t.ap().rearrange("(T tt i f) s -> T i tt (f s)",
                             T=NL2, tt=2, i=128, f=16)
    outv = out_t.ap().rearrange("(T tt i f) k -> T i tt (f k)",
                                T=NL2, tt=2, i=128, f=16)

    AF = mybir.ActivationFunctionType
    AL = mybir.AluOpType

    with tile.TileContext(nc) as tc:
        with tc.tile_pool(name="consts", bufs=1) as cp, \
             tc.tile_pool(name="persist", bufs=1) as pp, \
             tc.tile_pool(name="keep", bufs=1) as kp, \
             tc.tile_pool(name="dram", bufs=1, space="DRAM") as dp:

            w1bd = cp.tile([64, 128], F32)
            w1lo = cp.tile([128, 128], F32)
            w1hi = cp.tile([128, 128], F32)
            w2bd = cp.tile([128, 32], BF16)
            identb = cp.tile([128, 128], BF16)
            wc2 = cp.tile([128, NT2], F32)
            wc2b = cp.tile([128, NT2], BF16)
            onesb = cp.tile([128, 1], BF16)
            pcon = cp.tile([128, 4], F32)
            rcon = cp.tile([1, 32], F32)
            nc.sync.dma_start(out=w1bd[:], in_=w1bd_t.ap())
            nc.sync.dma_start(out=w1lo[:], in_=w1lo_t.ap())
            nc.sync.dma_start(out=w1hi[:], in_=w1hi_t.ap())
            nc.sync.dma_start(out=w2bd[:], in_=w2bd_t.ap())
            nc.sync.dma_start(out=identb[:], in_=identb_t.ap())
            nc.sync.dma_start(out=wc2[:], in_=wcol2_t.ap())
            nc.sync.dma_start(out=wc2b[:], in_=wc2b_t.ap())
            nc.sync.dma_start(out=onesb[:], in_=onesb_t.ap())
            nc.sync.dma_start(out=pcon[:], in_=pcon_t.ap())
            nc.sync.dma_start(out=rcon[:], in_=rcon_t.ap())

            pack = pp.tile([1, 64], F32)
            stats = pp.tile([1, 64], F32)
            sca = pp.tile([1, 128], F32)    # tile(scale, 8)
            scb = pp.tile([1, 64], F32)     # tile(shift, 4)
            scaleb8 = pp.tile([128, 1], F32)
            shiftb = pp.tile([64, 1], F32)
            w1lop = pp.tile([128, 128], BF16)
            w1hip = pp.tile([128, 128], BF16)
            b1p = pp.tile([128, 1], F32)
            st = pp.tile([1, 16], F32)
            st2 = pp.tile([1, 16], F32)
            st3 = pp.tile([1, 16], F32)
            rcnt = pp.tile([1, 1], F32)

            bncA = dp.tile([1, 64], F32)
            bncB = dp.tile([1, 64], F32)

            # retained bf16 shard (16.8 MB), one tile per paired load
            xk = [kp.tile([128, 512], BF16, tag=f"xk{L}", name=f"xk{L}")
                  for L in range(NL2)]

            # ============ pass 1: cast-load + masked stats ============
            with tc.tile_pool(name="p1s", bufs=2) as p1s, \
                 tc.tile_pool(name="p1p", bufs=1, space="PSUM") as p1p:
                psx = p1p.tile([1, 256], F32)
                psx2 = p1p.tile([1, 256], F32)
                pcnt = p1p.tile([1, 256], F32)
                for L in range(NL2):
                    # cast f32 -> bf16 during the (SWDGE) DMA
                    nc.gpsimd.dma_start(
                        out=xk[L][:].rearrange("p (tt g) -> p tt g",
                                               tt=2, g=256),
                        in_=xp2[L])
                    sq = p1s.tile([128, 512], BF16)
                    nc.vector.tensor_tensor(sq[:], xk[L][:], xk[L][:],
                                            op=AL.mult)
                    for tt in range(2):
                        t = 2 * L + tt
                        wcol = wc2b[:, t:t + 1]
                        nc.tensor.matmul(
                            psx[:], wcol, xk[L][:, 256 * tt:256 * tt + 256],
                            start=(t == 0), stop=(t == NT2 - 1),
                            skip_group_check=True)
                        nc.tensor.matmul(
                            psx2[:], wcol, sq[:, 256 * tt:256 * tt + 256],
                            start=(t == 0), stop=(t == NT2 - 1),
                            skip_group_check=True)
                nc.tensor.matmul(pcnt[:], onesb[:], wc2b[:],
                                 start=True, stop=True, skip_group_check=True)
                nc.vector.memset(pack[:], 0.0)
                # psum free layout f = 16*q + s: reduce over q keeping s
                nc.vector.tensor_reduce(
                    pack[0:1, 0:16],
                    psx.rearrange("p (q s) -> p s q", q=16, s=16),
                    axis=mybir.AxisListType.X, op=AL.add)
                nc.vector.tensor_reduce(
                    pack[0:1, 16:32],
                    psx2.rearrange("p (q s) -> p s q", q=16, s=16),
                    axis=mybir.AxisListType.X, op=AL.add)
                nc.vector.tensor_reduce(
                    pack[0:1, 32:33], pcnt[:],
                    axis=mybir.AxisListType.X, op=AL.add)

            # ====== prefetched pass-2 transposes (stats-independent) ======
            # these sit in the PE stream BEFORE any stats-dependent op, so
            # the PE keeps working through the all-reduce latency
            KPRE = 32
            pre_sxT = [pp.tile([128, 512], BF16, tag=f"psx{L}",
                               name=f"psx{L}") for L in range(KPRE)]
            with tc.tile_pool(name="prep", bufs=2, space="PSUM") as prep:
                for L in range(KPRE):
                    pxT = prep.tile([128, 512], BF16)
                    for q in range(4):
                        nc.tensor.transpose(pxT[:, 128 * q:128 * q + 128],
                                            xk[L][:, 128 * q:128 * q + 128],
                                            identb[:])
                    nc.vector.tensor_copy(pre_sxT[L][:], pxT[:])

            # ================= all-reduce + BN fold =================
            nc.gpsimd.dma_start(out=bncA[:], in_=pack[:])
            nc.gpsimd.collective_compute(
                "AllReduce", AL.add,
                replica_groups=[list(range(NCORES))],
                ins=[bncA.opt()], outs=[bncB.opt()])
            nc.gpsimd.dma_start(out=stats[:], in_=bncB[:])

            s_sx = stats[0:1, 0:16]
            s_sx2 = stats[0:1, 16:32]
            s_cnt = stats[0:1, 32:33]
            gam = rcon[0:1, 0:16]
            bet = rcon[0:1, 16:32]
            nc.vector.reciprocal(rcnt[:], s_cnt)
            nc.vector.tensor_scalar_mul(st[:], s_sx, rcnt[:])    # mean
            nc.vector.tensor_scalar_mul(st2[:], s_sx2, rcnt[:])  # E[x^2]
            nc.vector.tensor_tensor(st3[:], st[:], st[:], op=AL.mult)
            nc.vector.tensor_sub(st2[:], st2[:], st3[:])         # var
            nc.vector.tensor_scalar_add(st2[:], st2[:], float(EPS))
            nc.vector.reciprocal(st3[:], st2[:])
            nc.scalar.sqrt(st3[:], st3[:])                       # rstd
            nc.vector.tensor_tensor(st3[:], st3[:], gam, op=AL.mult)   # scale
            nc.vector.tensor_tensor(st2[:], st[:], st3[:], op=AL.mult)
            nc.vector.tensor_sub(st2[:], bet, st2[:])            # shift
            for g in range(8):
                nc.vector.tensor_copy(sca[0:1, 16 * g:16 * g + 16], st3[:])
            for g in range(4):
                nc.vector.tensor_copy(scb[0:1, 16 * g:16 * g + 16], st2[:])
            nc.sync.dma_start(out=scaleb8[:], in_=sca[:])
            nc.sync.dma_start(out=shiftb[:], in_=scb[:])
            nc.scalar.mul(w1lop[:], w1lo[:], mul=scaleb8[:])
            nc.scalar.mul(w1hip[:], w1hi[:], mul=scaleb8[:])
            with tc.tile_pool(name="stp", bufs=1, space="PSUM") as stp:
                pb1 = stp.tile([128, 1], F32)
                nc.tensor.matmul(pb1[:], w1bd[:], shiftb[:],
                                 start=True, stop=True, skip_group_check=True)
                nc.scalar.activation(b1p[:], pb1[:], AF.Identity,
                                     bias=pcon[:, 1:2], scale=1.0)

            # ================= pass 2: MLP =================
            # logical tile t = 2L+tt; sub-tile j = 2a+jj; rows
            # r = 2048t + 16i + 4j + g, chan s.
            with tc.tile_pool(name="p2st", bufs=3) as p2st, \
                 tc.tile_pool(name="p2sh", bufs=3) as p2sh, \
                 tc.tile_pool(name="p2sy", bufs=3) as p2sy, \
                 tc.tile_pool(name="p2so", bufs=3) as p2so, \
                 tc.tile_pool(name="p2pa", bufs=2, space="PSUM") as p2pa, \
                 tc.tile_pool(name="p2pb", bufs=2, space="PSUM") as p2pb, \
                 tc.tile_pool(name="p2pc", bufs=2, space="PSUM") as p2pc, \
                 tc.tile_pool(name="p2pd", bufs=2, space="PSUM") as p2pd:
                for L in range(NL2):
                    xab = xk[L]
                    if L < KPRE:
                        sxT = pre_sxT[L]
                    else:
                        # 4 transposes: quarter q=(tt,a) -> pxT[:,128q]
                        pxT = p2pa.tile([128, 512], BF16)
                        for q in range(4):
                            nc.tensor.transpose(
                                pxT[:, 128 * q:128 * q + 128],
                                xab[:, 128 * q:128 * q + 128],
                                identb[:])
                        sxT = p2st.tile([128, 512], BF16)
                        nc.vector.tensor_copy(sxT[:], pxT[:])
                    py = p2pc.tile([128, 256], F32)
                    for tt in range(2):
                        # layer 1: lo covers jj=0 (partitions 0:64), hi jj=1
                        ph = p2pb.tile([128, 512], F32, tag="ph")
                        rhs = sxT[:, 256 * tt:256 * tt + 256]
                        nc.tensor.matmul(ph[:, 0:256], w1lop[:], rhs,
                                         start=True, stop=True,
                                         skip_group_check=True)
                        nc.tensor.matmul(ph[:, 256:512], w1hip[:], rhs,
                                         start=True, stop=True,
                                         skip_group_check=True)
                        sh = p2sh.tile([128, 512], BF16, tag="sh")
                        nc.scalar.activation(sh[:], ph[:], AF.Relu,
                                             bias=b1p[:], scale=1.0)
                        # layer 2: sub-tile j=2a+jj at sh[:, 256jj+128a]
                        for jj in range(2):
                            for a in range(2):
                                j = 2 * a + jj
                                nc.tensor.matmul(
                                    py[32 * j:32 * j + 32,
                                       128 * tt:128 * tt + 128],
                                    w2bd[:],
                                    sh[:, 256 * jj + 128 * a:
                                       256 * jj + 128 * a + 128],
                                    start=True, stop=True,
                                    skip_group_check=True,
                                    tile_position=(0, 32 * j))
                    sy = p2sy.tile([128, 256], BF16)
                    nc.scalar.activation(sy[:], py[:], AF.Identity,
                                         bias=pcon[:, 2:3], scale=1.0)
                    pyr = p2pd.tile([128, 256], BF16)
                    for u in range(2):
                        nc.tensor.transpose(pyr[:, 128 * u:128 * u + 128],
                                            sy[:, 128 * u:128 * u + 128],
                                            identb[:])
                    so = p2so.tile([128, 256], F32)
                    for u in range(2):
                        nc.vector.tensor_scalar_mul(
                            so[:, 128 * u:128 * u + 128],
                            pyr[:, 128 * u:128 * u + 128],
                            wc2[:, 2 * L + u:2 * L + u + 1])
                    nc.sync.dma_start(
                        out=outv[L],
                        in_=so[:].rearrange("p (tt g) -> p tt g",
                                            tt=2, g=128))

    nc.compile()
    return nc


_NC = None


def _get_nc():
    global _NC
    if _NC is None:
        _NC = build()
    return _NC


def _prep_inputs(stacks, mask, gamma, beta, W1, b1, W2, b2):
    stacks = np.ascontiguousarray(np.asarray(stacks, dtype=np.float32))
    mask = np.asarray(mask)
    w_full = (~mask.astype(bool)).astype(np.float32)
    gamma = np.asarray(gamma, np.float32)
    beta = np.asarray(beta, np.float32)
    W1 = np.asarray(W1, np.float32)
    b1 = np.asarray(b1, np.float32)
    W2 = np.asarray(W2, np.float32)
    b2 = np.asarray(b2, np.float32)

    w1bd = np.zeros((64, 128), np.float32)
    w1lo = np.zeros((128, 128), np.float32)
    w1hi = np.zeros((128, 128), np.float32)
    w2bd = np.zeros((128, 32), np.float32)
    for g in range(4):
        w1bd[16 * g:16 * g + 16, 32 * g:32 * g + 32] = W1
        w1lo[16 * g:16 * g + 16, 32 * g:32 * g + 32] = W1
        w1hi[64 + 16 * g:64 + 16 * g + 16, 32 * g:32 * g + 32] = W1
        w2bd[32 * g:32 * g + 32, 8 * g:8 * g + 8] = W2
    identb = np.eye(128).astype(BF_NP)
    onesb = np.full((128, 1), 16.0, BF_NP)   # 16 rows per wcol2 entry
    pcon = np.zeros((128, 4), np.float32)
    pcon[:, 1] = np.tile(b1, 4)
    pcon[:, 2] = np.tile(b2, 16)
    rcon = np.zeros((1, 32), np.float32)
    rcon[0, 0:16] = gamma
    rcon[0, 16:32] = beta

    I2 = np.arange(128) // 16
    T2 = 8 * np.arange(NT2)

    in_maps = []
    for c in range(NCORES):
        shard = stacks[c * BL:(c + 1) * BL].reshape(R, S)
        w = w_full[c * BL:(c + 1) * BL].reshape(BL * NN)
        wcol2 = np.ascontiguousarray(
            w[I2[:, None] + T2[None, :]].astype(np.float32))
        in_maps.append({
            "stacks": shard,
            "wcol2": wcol2, "wc2b": wcol2.astype(BF_NP), "onesb": onesb,
            "w1bd": w1bd, "w1lo": w1lo, "w1hi": w1hi,
            "w2bd": w2bd.astype(BF_NP), "identb": identb,
            "pcon": pcon, "rcon": rcon,
        })
    return in_maps


def _run(inputs, trace=False, tmpdir=None):
    nc = _get_nc()
    in_maps = _prep_inputs(**inputs)
    res = run_bass_kernel_spmd(nc, in_maps, core_ids=list(range(NCORES)),
                               trace=trace, tmpdir=tmpdir)
    outs = [res.results[c]["out"].reshape(BL, NN, NN, HEADS)
            for c in range(NCORES)]
    full = np.concatenate(outs, axis=0).astype(np.float32)
    return full, res


def kernel(**inputs):
    out, _ = _run(inputs, trace=False)
    return out
